# revision 2
# baseline (speedup 1.0000x reference)
"""Trainium2 Bass kernel for nn_MixedActivation.

Column i of x uses activation (i % 6): 0,1,2 -> square; 3,4,5 -> PReLU with
prelu_a[0..2]. Data-parallel over rows across 8 NeuronCores (125000 rows
each); the PReLU scalars are baked into each core's program as immediates.

The kernel is DMA/bandwidth-bound: with the 2e-2 relative-error budget the
tensor travels as bf16 both ways (rel err ~1.1e-2, dominated by the squared
columns), i.e. 12 MB in + 12 MB out per core. Measured combined throughput
tops out at ~410 GB/s per core (94% of the 435 GB/s SBUF-AXI fabric
ceiling), so the DMA span floor is ~58.5 us; runtime preamble (~6 us),
first-byte latency and the final HBM write receipt add ~10 us of fixed
overhead. Best measured: ~69.1 us.

Layout ("flat"): the shard is treated as a flat 6,000,000-element stream.
Every partition carries exactly E = 46872 contiguous elements (E % 6 == 0,
so the mod-6 column phase is identical in every partition), split into 9
uniform tiles [128, 5208] plus one [1, 384] leftover on partition 0. This
balances the 16 SDMA engines exactly (the old row-aligned tiling ended with
a 98-partition tail tile that idled 30 partitions' worth of engine
capacity). The whole shard is SBUF-resident (94.5 KB/partition), so slots
are single-use and no write-after-read hazards exist.

Schedule: SP issues all in-DMAs up front and all out-DMAs (one HWDGE ring,
FIFO keeps the engines fed back-to-back); the second big load is issued by
ACT on its own ring so the first two descriptor-gens overlap. DVE squares
phases 0-2 in place (one strided run-3 tensor_tensor per tile); ACT applies
Prelu to phases 3-5 (one strided run-3 activation per tile -- the three
alphas are equal in the reference; unequal alphas fall back to per-phase
instructions). A dummy 8-element activation at program start hoists ACT's
one-time function-table load off tile 0's critical path. DMAs are bitcast
to uint32 (identical bytes, 4-byte descriptor costing). Per-work in-sems
keep load-completion counts exact; compute sems gate each out-DMA; a single
cumulative out-sem gates program end.
"""

import numpy as np

import concourse.bass as bass
import concourse.mybir as mybir
from concourse.bass_utils import run_bass_kernel_spmd

N_CORES = 8
ROWS = 1_000_000
COLS = 48
SHARD_ROWS = ROWS // N_CORES  # 125000

P = 128


def _build_flat(prelu_a, NT=9, act_warm=True):
    """Optimized builder: flat phase-aligned layout, perfectly balanced DMA."""
    TOT = SHARD_ROWS * COLS            # 6,000,000 elements
    E = (TOT // P) // 6 * 6            # 46872 per partition, mod 6 == 0
    LEFT = TOT - P * E                 # 384 elements, on partition 0
    assert E % NT == 0
    F = E // NT                        # 5208 elements per tile per partition
    G = F // 6
    GL = LEFT // 6

    a0, a1, a2 = (float(v) for v in prelu_a)
    fused = a0 == a1 == a2

    _orig_preamble = bass.BassEngine.preamble
    bass.BassEngine.preamble = lambda self: None
    try:
        nc = bass.Bass("TRN2", target_bir_lowering=False)
    finally:
        bass.BassEngine.preamble = _orig_preamble

    x_ext = nc.declare_dram_parameter(
        "x", [SHARD_ROWS, COLS], mybir.dt.bfloat16, isOutput=False
    )
    y_ext = nc.declare_dram_parameter(
        "y", [SHARD_ROWS, COLS], mybir.dt.bfloat16, isOutput=True
    )
    x_flat = x_ext.rearrange("r c -> (r c)")
    y_flat = y_ext.rearrange("r c -> (r c)")
    x_main = x_flat[0 : P * E].rearrange("(p e) -> p e", p=P, e=E)
    y_main = y_flat[0 : P * E].rearrange("(p e) -> p e", p=P, e=E)
    x_left = x_flat[P * E : TOT].rearrange("(p e) -> p e", p=1, e=LEFT)
    y_left = y_flat[P * E : TOT].rearrange("(p e) -> p e", p=1, e=LEFT)

    from contextlib import ExitStack

    with ExitStack() as stack:
        tin = stack.enter_context(
            nc.sbuf_tensor([P, E + LEFT], mybir.dt.bfloat16)
        )
        if act_warm:
            warm = stack.enter_context(nc.sbuf_tensor([1, 8], mybir.dt.bfloat16))
        NW = NT + 1  # work 0 = leftover, 1..NT = main tiles
        in_sems = [
            stack.enter_context(nc.semaphore(f"in_sem{i}")) for i in range(NW)
        ]
        out_sem = stack.enter_context(nc.semaphore("out_sem"))
        sq_sem = stack.enter_context(nc.semaphore("sq_sem"))
        pr_sem = stack.enter_context(nc.semaphore("pr_sem"))
        block = stack.enter_context(nc.Block())

        def din(i):
            return x_left if i == 0 else x_main[:, (i - 1) * F : i * F]

        def dout(i):
            return y_left if i == 0 else y_main[:, (i - 1) * F : i * F]

        def buf(i):
            if i == 0:
                return tin[0:1, E : E + LEFT]
            return tin[:, (i - 1) * F : i * F]

        # Load order: tile 1 first (engines spin up on a big transfer),
        # tiny leftover second, rest after. Tile 2's load goes out on ACT's
        # HWDGE ring so the first two big descriptor-gens overlap.
        sp_loads = [1, 0] + list(range(3, NW))
        act_loads = [2] if NW > 2 else []

        @block.sync
        def _(sync):
            for i in sp_loads:
                sync.dma_start(
                    out=buf(i).bitcast(mybir.dt.uint32),
                    in_=din(i).bitcast(mybir.dt.uint32),
                ).then_inc(in_sems[i], 16)
            for i in range(NW):
                sync.wait_ge(sq_sem, i + 1)
                sync.wait_ge(pr_sem, i + 1)
                sync.dma_start(
                    out=dout(i).bitcast(mybir.dt.uint32),
                    in_=buf(i).bitcast(mybir.dt.uint32),
                ).then_inc(out_sem, 16)
            sync.wait_ge(out_sem, 16 * NW)

        @block.vector
        def _(vector):
            for i in range(NW):
                vector.wait_ge(in_sems[i], 16)
                g = GL if i == 0 else G
                v = buf(i).rearrange("p (g s) -> p g s", g=g, s=6)
                vector.tensor_tensor(
                    out=v[:, :, 0:3],
                    in0=v[:, :, 0:3],
                    in1=v[:, :, 0:3],
                    op=mybir.AluOpType.mult,
                )
                vector.drain().then_inc(sq_sem, 1)

        @block.scalar
        def _(scalar):
            for i in act_loads:
                scalar.dma_start(
                    out=buf(i).bitcast(mybir.dt.uint32),
                    in_=din(i).bitcast(mybir.dt.uint32),
                ).then_inc(in_sems[i], 16)
            if act_warm:
                scalar.activation(
                    out=warm[:, :],
                    in_=warm[:, :],
                    func=mybir.ActivationFunctionType.Prelu,
                    alpha=a0,
                )
            for i in range(NW):
                scalar.wait_ge(in_sems[i], 16)
                g = GL if i == 0 else G
                v = buf(i).rearrange("p (g s) -> p g s", g=g, s=6)
                if fused:
                    scalar.activation(
                        out=v[:, :, 3:6],
                        in_=v[:, :, 3:6],
                        func=mybir.ActivationFunctionType.Prelu,
                        alpha=a0,
                    )
                else:
                    for k, a in enumerate((a0, a1, a2)):
                        scalar.activation(
                            out=v[:, :, 3 + k : 4 + k],
                            in_=v[:, :, 3 + k : 4 + k],
                            func=mybir.ActivationFunctionType.Prelu,
                            alpha=a,
                        )
                scalar.drain().then_inc(pr_sem, 1)

    return nc


# ---------------------------------------------------------------------------
# Legacy row-aligned tiled builder. Kept only for test.py's K-replica
# differencing fallback (replicas > 1 unrolls the pipeline for slope timing);
# the graded kernel() uses _build_flat above.
# ---------------------------------------------------------------------------

B = 100                 # rows per partition per tile
TILE_ROWS = P * B       # 12800
NB = 10                 # buffer slots; 10 = whole shard resident, no WAR
F = COLS * B            # 4800 elements per partition


def _build(prelu_a, replicas=1, B=B, NB=NB):
    TILE_ROWS = P * B
    N_FULL = SHARD_ROWS // TILE_ROWS
    TAIL_ROWS = SHARD_ROWS - N_FULL * TILE_ROWS
    assert TAIL_ROWS % B == 0
    TAIL_P = TAIL_ROWS // B
    NTILES = N_FULL + (1 if TAIL_ROWS else 0)
    F = COLS * B
    a0, a1, a2 = (float(v) for v in prelu_a)
    if a0 == a1 == a2:
        mode = "fused"
    elif 0.0 <= a2 <= 1.0:
        mode = "split"
    else:
        mode = "generic"
    _orig_preamble = bass.BassEngine.preamble
    bass.BassEngine.preamble = lambda self: None
    try:
        nc = bass.Bass("TRN2", target_bir_lowering=False)
    finally:
        bass.BassEngine.preamble = _orig_preamble
    x_ext = nc.declare_dram_parameter(
        "x", [SHARD_ROWS, COLS], mybir.dt.bfloat16, isOutput=False
    )
    y_ext = nc.declare_dram_parameter(
        "y", [SHARD_ROWS, COLS], mybir.dt.bfloat16, isOutput=True
    )

    x_full = x_ext[0 : N_FULL * TILE_ROWS, :].rearrange(
        "(n p b) c -> n p (b c)", n=N_FULL, p=P, b=B
    )
    y_full = y_ext[0 : N_FULL * TILE_ROWS, :].rearrange(
        "(n p b) c -> n p (b c)", n=N_FULL, p=P, b=B
    )
    if TAIL_ROWS:
        x_tail = x_ext[N_FULL * TILE_ROWS :, :].rearrange(
            "(p b) c -> p (b c)", p=TAIL_P, b=B
        )
        y_tail = y_ext[N_FULL * TILE_ROWS :, :].rearrange(
            "(p b) c -> p (b c)", p=TAIL_P, b=B
        )

    def dram_in(i):
        return x_full[i] if i < N_FULL else x_tail

    def dram_out(i):
        return y_full[i] if i < N_FULL else y_tail

    def pdim(i):
        return P if i < N_FULL else TAIL_P

    from contextlib import ExitStack

    with ExitStack() as stack:
        tin = stack.enter_context(
            nc.sbuf_tensor([P, NB * F], mybir.dt.bfloat16)
        )
        in_sems = [
            stack.enter_context(nc.semaphore(f"in_sem{b}")) for b in range(NB)
        ]
        out_sems = [
            stack.enter_context(nc.semaphore(f"out_sem{b}")) for b in range(NB)
        ]
        sq_sem = stack.enter_context(nc.semaphore("sq_sem"))
        pr_sem = stack.enter_context(nc.semaphore("pr_sem"))
        block = stack.enter_context(nc.Block())

        NT = NTILES * replicas

        def dti(t):
            return t % NTILES

        def buf(t):
            return tin[: pdim(dti(t)), (t % NB) * F : (t % NB + 1) * F]

        def n_loads(t):
            return 16 * (t // NB + 1)

        @block.sync
        def _(sync):
            for t in range(NT):
                if t >= NB:
                    sync.wait_ge(out_sems[t % NB], n_loads(t - NB))
                sync.dma_start(
                    out=buf(t).bitcast(mybir.dt.uint32),
                    in_=dram_in(dti(t)).bitcast(mybir.dt.uint32),
                ).then_inc(in_sems[t % NB], 16)
            for b in range(min(NB, NT)):
                last_t = NT - 1 - (NT - 1 - b) % NB
                sync.wait_ge(out_sems[b], n_loads(last_t))

        @block.vector
        def _(vector):
            for t in range(NT):
                vector.wait_ge(in_sems[t % NB], n_loads(t))
                v = buf(t).rearrange("p (b g s) -> p b g s", b=B, g=8, s=6)
                vector.tensor_tensor(
                    out=v[:, :, :, 0:3],
                    in0=v[:, :, :, 0:3],
                    in1=v[:, :, :, 0:3],
                    op=mybir.AluOpType.mult,
                )
                if mode == "split":
                    vector.scalar_tensor_tensor(
                        out=v[:, :, :, 5:6],
                        in0=v[:, :, :, 5:6],
                        scalar=a2,
                        in1=v[:, :, :, 5:6],
                        op0=mybir.AluOpType.mult,
                        op1=mybir.AluOpType.max,
                    )
                vector.drain().then_inc(sq_sem, 1)

        @block.scalar
        def _(scalar):
            for t in range(NT):
                i = dti(t)
                scalar.wait_ge(in_sems[t % NB], n_loads(t))
                v = buf(t).rearrange("p (b g s) -> p b g s", b=B, g=8, s=6)
                if mode == "fused":
                    scalar.activation(
                        out=v[:, :, :, 3:6],
                        in_=v[:, :, :, 3:6],
                        func=mybir.ActivationFunctionType.Prelu,
                        alpha=a0,
                    )
                else:
                    nk = 2 if mode == "split" else 3
                    for k, a in list(enumerate((a0, a1, a2)))[:nk]:
                        scalar.activation(
                            out=v[:, :, :, 3 + k : 4 + k],
                            in_=v[:, :, :, 3 + k : 4 + k],
                            func=mybir.ActivationFunctionType.Prelu,
                            alpha=a,
                        )
                scalar.drain().then_inc(pr_sem, 1)
                scalar.wait_ge(sq_sem, t + 1)
                scalar.dma_start(
                    out=dram_out(i).bitcast(mybir.dt.uint32),
                    in_=buf(t).bitcast(mybir.dt.uint32),
                ).then_inc(out_sems[t % NB], 16)

    return nc


def kernel(x: np.ndarray, prelu_a: np.ndarray, trace: bool = False):
    import ml_dtypes

    nc = _build_flat(prelu_a)
    xb = np.ascontiguousarray(x, dtype=np.float32).astype(ml_dtypes.bfloat16)
    in_maps = [
        {"x": xb[c * SHARD_ROWS : (c + 1) * SHARD_ROWS]} for c in range(N_CORES)
    ]
    res = run_bass_kernel_spmd(nc, in_maps, list(range(N_CORES)), trace=trace)
    out = np.concatenate(
        [np.asarray(res.results[c]["y"]).astype(np.float32) for c in range(N_CORES)],
        axis=0,
    )
    if trace:
        return out, res
    return out


# revision 4
# speedup vs baseline: 1.0320x; 1.0320x over previous
"""Trainium2 Bass kernel for nn_MixedActivation.

Column i of x uses activation (i % 6): 0,1,2 -> square; 3,4,5 -> PReLU with
prelu_a[0..2]. Data-parallel over rows across 8 NeuronCores (125000 rows
each); the PReLU scalars are baked into each core's program as immediates.

The kernel is DMA/bandwidth-bound: with the 2e-2 relative-error budget the
tensor travels as bf16 both ways (rel err ~1.1e-2, dominated by the squared
columns), i.e. 12 MB in + 12 MB out per core. Measured combined throughput
tops out at ~410 GB/s per core (94% of the 435 GB/s SBUF-AXI fabric
ceiling), so the DMA span floor is ~58.5 us; runtime preamble (~6 us),
first-byte latency and the final HBM write receipt add ~10 us of fixed
overhead. Best measured: ~69.1 us.

Layout ("flat"): the shard is treated as a flat 6,000,000-element stream.
Every partition carries exactly E = 46872 contiguous elements (E % 6 == 0,
so the mod-6 column phase is identical in every partition), split into 9
uniform tiles [128, 5208] plus one [1, 384] leftover on partition 0. This
balances the 16 SDMA engines exactly (the old row-aligned tiling ended with
a 98-partition tail tile that idled 30 partitions' worth of engine
capacity). The whole shard is SBUF-resident (94.5 KB/partition), so slots
are single-use and no write-after-read hazards exist.

Schedule: SP issues all in-DMAs up front and all out-DMAs (one HWDGE ring,
FIFO keeps the engines fed back-to-back); the second big load is issued by
ACT on its own ring so the first two descriptor-gens overlap. DVE squares
phases 0-2 in place (one strided run-3 tensor_tensor per tile); ACT applies
Prelu to phases 3-5 (one strided run-3 activation per tile -- the three
alphas are equal in the reference; unequal alphas fall back to per-phase
instructions). A dummy 8-element activation at program start hoists ACT's
one-time function-table load off tile 0's critical path. DMAs are bitcast
to uint32 (identical bytes, 4-byte descriptor costing). Per-work in-sems
keep load-completion counts exact; compute sems gate each out-DMA; a single
cumulative out-sem gates program end.
"""

import numpy as np

import concourse.bass as bass
import concourse.mybir as mybir
from concourse.bass_utils import run_bass_kernel_spmd

N_CORES = 8
ROWS = 1_000_000
COLS = 48
SHARD_ROWS = ROWS // N_CORES  # 125000

P = 128


def _build_flat(prelu_a, NT=9, act_warm=True):
    """Optimized builder: flat phase-aligned layout, perfectly balanced DMA."""
    TOT = SHARD_ROWS * COLS            # 6,000,000 elements
    E = (TOT // P) // 6 * 6            # 46872 per partition, mod 6 == 0
    LEFT = TOT - P * E                 # 384 elements, on partition 0
    assert E % NT == 0
    F = E // NT                        # 5208 elements per tile per partition
    G = F // 6
    GL = LEFT // 6

    a0, a1, a2 = (float(v) for v in prelu_a)
    # 'fused': equal alphas (the reference case) -> one run-3 Prelu on ACT.
    # 'split': phase 5 on DVE as max(a2*x, x) (valid for 0 <= a2 <= 1),
    #          phases 3,4 on ACT. 'generic': 3 Prelus on ACT.
    if a0 == a1 == a2:
        mode = "fused"
    elif 0.0 <= a2 <= 1.0:
        mode = "split"
    else:
        mode = "generic"

    _orig_preamble = bass.BassEngine.preamble
    bass.BassEngine.preamble = lambda self: None
    try:
        nc = bass.Bass("TRN2", target_bir_lowering=False)
    finally:
        bass.BassEngine.preamble = _orig_preamble

    x_ext = nc.declare_dram_parameter(
        "x", [SHARD_ROWS, COLS], mybir.dt.bfloat16, isOutput=False
    )
    y_ext = nc.declare_dram_parameter(
        "y", [SHARD_ROWS, COLS], mybir.dt.bfloat16, isOutput=True
    )
    x_flat = x_ext.rearrange("r c -> (r c)")
    y_flat = y_ext.rearrange("r c -> (r c)")
    x_main = x_flat[0 : P * E].rearrange("(p e) -> p e", p=P, e=E)
    y_main = y_flat[0 : P * E].rearrange("(p e) -> p e", p=P, e=E)
    x_left = x_flat[P * E : TOT].rearrange("(p e) -> p e", p=1, e=LEFT)
    y_left = y_flat[P * E : TOT].rearrange("(p e) -> p e", p=1, e=LEFT)

    from contextlib import ExitStack

    with ExitStack() as stack:
        tin = stack.enter_context(
            nc.sbuf_tensor([P, E + LEFT], mybir.dt.bfloat16)
        )
        if act_warm:
            warm = stack.enter_context(nc.sbuf_tensor([1, 8], mybir.dt.bfloat16))
        NW = NT + 1  # work 0 = leftover, 1..NT = main tiles
        in_sems = [
            stack.enter_context(nc.semaphore(f"in_sem{i}")) for i in range(NW)
        ]
        out_sem = stack.enter_context(nc.semaphore("out_sem"))
        sq_sem = stack.enter_context(nc.semaphore("sq_sem"))
        pr_sem = stack.enter_context(nc.semaphore("pr_sem"))
        block = stack.enter_context(nc.Block())

        def din(i):
            return x_left if i == 0 else x_main[:, (i - 1) * F : i * F]

        def dout(i):
            return y_left if i == 0 else y_main[:, (i - 1) * F : i * F]

        def buf(i):
            if i == 0:
                return tin[0:1, E : E + LEFT]
            return tin[:, (i - 1) * F : i * F]

        # Load order: tile 1 first (engines spin up on a big transfer),
        # tiny leftover second, rest after. Tile 2's load goes out on ACT's
        # HWDGE ring so the first two big descriptor-gens overlap.
        sp_loads = [1, 0] + list(range(3, NW))
        act_loads = [2] if NW > 2 else []

        @block.sync
        def _(sync):
            for i in sp_loads:
                sync.dma_start(
                    out=buf(i).bitcast(mybir.dt.uint32),
                    in_=din(i).bitcast(mybir.dt.uint32),
                ).then_inc(in_sems[i], 16)
            for i in range(NW):
                sync.wait_ge(sq_sem, i + 1)
                sync.wait_ge(pr_sem, i + 1)
                sync.dma_start(
                    out=dout(i).bitcast(mybir.dt.uint32),
                    in_=buf(i).bitcast(mybir.dt.uint32),
                ).then_inc(out_sem, 16)
            sync.wait_ge(out_sem, 16 * NW)

        @block.vector
        def _(vector):
            for i in range(NW):
                vector.wait_ge(in_sems[i], 16)
                g = GL if i == 0 else G
                v = buf(i).rearrange("p (g s) -> p g s", g=g, s=6)
                vector.tensor_tensor(
                    out=v[:, :, 0:3],
                    in0=v[:, :, 0:3],
                    in1=v[:, :, 0:3],
                    op=mybir.AluOpType.mult,
                )
                if mode == "split":
                    # prelu(x) = max(a*x, x) for 0 <= a <= 1
                    vector.scalar_tensor_tensor(
                        out=v[:, :, 5:6],
                        in0=v[:, :, 5:6],
                        scalar=a2,
                        in1=v[:, :, 5:6],
                        op0=mybir.AluOpType.mult,
                        op1=mybir.AluOpType.max,
                    )
                vector.drain().then_inc(sq_sem, 1)

        @block.scalar
        def _(scalar):
            for i in act_loads:
                scalar.dma_start(
                    out=buf(i).bitcast(mybir.dt.uint32),
                    in_=din(i).bitcast(mybir.dt.uint32),
                ).then_inc(in_sems[i], 16)
            if act_warm:
                scalar.activation(
                    out=warm[:, :],
                    in_=warm[:, :],
                    func=mybir.ActivationFunctionType.Prelu,
                    alpha=a0,
                )
            for i in range(NW):
                scalar.wait_ge(in_sems[i], 16)
                g = GL if i == 0 else G
                v = buf(i).rearrange("p (g s) -> p g s", g=g, s=6)
                if mode == "fused":
                    scalar.activation(
                        out=v[:, :, 3:6],
                        in_=v[:, :, 3:6],
                        func=mybir.ActivationFunctionType.Prelu,
                        alpha=a0,
                    )
                else:
                    nk = 2 if mode == "split" else 3
                    for k, a in list(enumerate((a0, a1, a2)))[:nk]:
                        scalar.activation(
                            out=v[:, :, 3 + k : 4 + k],
                            in_=v[:, :, 3 + k : 4 + k],
                            func=mybir.ActivationFunctionType.Prelu,
                            alpha=a,
                        )
                scalar.drain().then_inc(pr_sem, 1)

    return nc


# ---------------------------------------------------------------------------
# Legacy row-aligned tiled builder. Kept only for test.py's K-replica
# differencing fallback (replicas > 1 unrolls the pipeline for slope timing);
# the graded kernel() uses _build_flat above.
# ---------------------------------------------------------------------------

B = 100                 # rows per partition per tile
TILE_ROWS = P * B       # 12800
NB = 10                 # buffer slots; 10 = whole shard resident, no WAR
F = COLS * B            # 4800 elements per partition


def _build(prelu_a, replicas=1, B=B, NB=NB):
    TILE_ROWS = P * B
    N_FULL = SHARD_ROWS // TILE_ROWS
    TAIL_ROWS = SHARD_ROWS - N_FULL * TILE_ROWS
    assert TAIL_ROWS % B == 0
    TAIL_P = TAIL_ROWS // B
    NTILES = N_FULL + (1 if TAIL_ROWS else 0)
    F = COLS * B
    a0, a1, a2 = (float(v) for v in prelu_a)
    if a0 == a1 == a2:
        mode = "fused"
    elif 0.0 <= a2 <= 1.0:
        mode = "split"
    else:
        mode = "generic"
    _orig_preamble = bass.BassEngine.preamble
    bass.BassEngine.preamble = lambda self: None
    try:
        nc = bass.Bass("TRN2", target_bir_lowering=False)
    finally:
        bass.BassEngine.preamble = _orig_preamble
    x_ext = nc.declare_dram_parameter(
        "x", [SHARD_ROWS, COLS], mybir.dt.bfloat16, isOutput=False
    )
    y_ext = nc.declare_dram_parameter(
        "y", [SHARD_ROWS, COLS], mybir.dt.bfloat16, isOutput=True
    )

    x_full = x_ext[0 : N_FULL * TILE_ROWS, :].rearrange(
        "(n p b) c -> n p (b c)", n=N_FULL, p=P, b=B
    )
    y_full = y_ext[0 : N_FULL * TILE_ROWS, :].rearrange(
        "(n p b) c -> n p (b c)", n=N_FULL, p=P, b=B
    )
    if TAIL_ROWS:
        x_tail = x_ext[N_FULL * TILE_ROWS :, :].rearrange(
            "(p b) c -> p (b c)", p=TAIL_P, b=B
        )
        y_tail = y_ext[N_FULL * TILE_ROWS :, :].rearrange(
            "(p b) c -> p (b c)", p=TAIL_P, b=B
        )

    def dram_in(i):
        return x_full[i] if i < N_FULL else x_tail

    def dram_out(i):
        return y_full[i] if i < N_FULL else y_tail

    def pdim(i):
        return P if i < N_FULL else TAIL_P

    from contextlib import ExitStack

    with ExitStack() as stack:
        tin = stack.enter_context(
            nc.sbuf_tensor([P, NB * F], mybir.dt.bfloat16)
        )
        in_sems = [
            stack.enter_context(nc.semaphore(f"in_sem{b}")) for b in range(NB)
        ]
        out_sems = [
            stack.enter_context(nc.semaphore(f"out_sem{b}")) for b in range(NB)
        ]
        sq_sem = stack.enter_context(nc.semaphore("sq_sem"))
        pr_sem = stack.enter_context(nc.semaphore("pr_sem"))
        block = stack.enter_context(nc.Block())

        NT = NTILES * replicas

        def dti(t):
            return t % NTILES

        def buf(t):
            return tin[: pdim(dti(t)), (t % NB) * F : (t % NB + 1) * F]

        def n_loads(t):
            return 16 * (t // NB + 1)

        @block.sync
        def _(sync):
            for t in range(NT):
                if t >= NB:
                    sync.wait_ge(out_sems[t % NB], n_loads(t - NB))
                sync.dma_start(
                    out=buf(t).bitcast(mybir.dt.uint32),
                    in_=dram_in(dti(t)).bitcast(mybir.dt.uint32),
                ).then_inc(in_sems[t % NB], 16)
            for b in range(min(NB, NT)):
                last_t = NT - 1 - (NT - 1 - b) % NB
                sync.wait_ge(out_sems[b], n_loads(last_t))

        @block.vector
        def _(vector):
            for t in range(NT):
                vector.wait_ge(in_sems[t % NB], n_loads(t))
                v = buf(t).rearrange("p (b g s) -> p b g s", b=B, g=8, s=6)
                vector.tensor_tensor(
                    out=v[:, :, :, 0:3],
                    in0=v[:, :, :, 0:3],
                    in1=v[:, :, :, 0:3],
                    op=mybir.AluOpType.mult,
                )
                if mode == "split":
                    vector.scalar_tensor_tensor(
                        out=v[:, :, :, 5:6],
                        in0=v[:, :, :, 5:6],
                        scalar=a2,
                        in1=v[:, :, :, 5:6],
                        op0=mybir.AluOpType.mult,
                        op1=mybir.AluOpType.max,
                    )
                vector.drain().then_inc(sq_sem, 1)

        @block.scalar
        def _(scalar):
            for t in range(NT):
                i = dti(t)
                scalar.wait_ge(in_sems[t % NB], n_loads(t))
                v = buf(t).rearrange("p (b g s) -> p b g s", b=B, g=8, s=6)
                if mode == "fused":
                    scalar.activation(
                        out=v[:, :, :, 3:6],
                        in_=v[:, :, :, 3:6],
                        func=mybir.ActivationFunctionType.Prelu,
                        alpha=a0,
                    )
                else:
                    nk = 2 if mode == "split" else 3
                    for k, a in list(enumerate((a0, a1, a2)))[:nk]:
                        scalar.activation(
                            out=v[:, :, :, 3 + k : 4 + k],
                            in_=v[:, :, :, 3 + k : 4 + k],
                            func=mybir.ActivationFunctionType.Prelu,
                            alpha=a,
                        )
                scalar.drain().then_inc(pr_sem, 1)
                scalar.wait_ge(sq_sem, t + 1)
                scalar.dma_start(
                    out=dram_out(i).bitcast(mybir.dt.uint32),
                    in_=buf(t).bitcast(mybir.dt.uint32),
                ).then_inc(out_sems[t % NB], 16)

    return nc


def kernel(x: np.ndarray, prelu_a: np.ndarray, trace: bool = False):
    import ml_dtypes

    nc = _build_flat(prelu_a)
    xb = np.ascontiguousarray(x, dtype=np.float32).astype(ml_dtypes.bfloat16)
    in_maps = [
        {"x": xb[c * SHARD_ROWS : (c + 1) * SHARD_ROWS]} for c in range(N_CORES)
    ]
    res = run_bass_kernel_spmd(nc, in_maps, list(range(N_CORES)), trace=trace)
    out = np.concatenate(
        [np.asarray(res.results[c]["y"]).astype(np.float32) for c in range(N_CORES)],
        axis=0,
    )
    if trace:
        return out, res
    return out
